# revision 2
# baseline (speedup 1.0000x reference)
"""Multi-head causal self-attention (B=4, T=2048, C=1024, H=16) on 8 TRN2 cores.

Sharding: core c handles batch b = c//2 and head-group hg = c%2 (8 heads):
data parallel over B, tensor parallel over H. Each core computes qk^T for its
heads, V in natural layout, causal attention for its 8 heads, and a partial
output projection (row-split W_proj) -> y_partial [T, C]. Host sums
y[b] = y_partial[2b] + y_partial[2b+1] + b_proj.

v2 over the original:
- q/k stored pair-packed [chA(64) | chB(64), T]; scores via two concurrent
  K=64 PE row-tiled matmuls (tile_position (0,0)/(64,0)) writing adjacent
  PSUM banks -> ~2x score throughput, no zero-padding memsets.
- exp batched over both heads' scores ([128,1024] PSUM read per ACTIVATE).
- score matmuls batched in groups of 4 kb-blocks to amortize PE tiling-mode
  switches vs the 128-row attnV matmuls.
- qkv projection for the second token half is emitted interleaved into
  attention round qc=1 (engines execute in-order; emission order = overlap).
- attention outputs kept in bf16 (aoT) and projection runs bf16 x bf16.
"""

from contextlib import ExitStack

import ml_dtypes
import numpy as np

import concourse.bass as bass
import concourse.bacc as bacc
import concourse.mybir as mybir
import concourse.tile as tile
from concourse.bass_utils import run_bass_kernel_spmd
from concourse.masks import make_upper_triangular

B, T, C, H, HS = 4, 2048, 1024, 16, 64
P = 128
NQC = T // 512          # q-chunks of 512
NKB = T // P            # key blocks of 128
TH = T // 2             # t-half
SCALE = HS ** -0.5

F32 = mybir.dt.float32
F32R = mybir.dt.float32r
BF16 = mybir.dt.bfloat16
Exp = mybir.ActivationFunctionType.Exp


def build_kernel():
    nc = bacc.Bacc("TRN2", target_bir_lowering=False)

    xt_d = nc.dram_tensor("xt", (C, T), BF16, kind="ExternalInput")
    wqk_d = nc.dram_tensor("wqk", (C, 8 * P), BF16, kind="ExternalInput")
    bqk_d = nc.dram_tensor("bqk", (8 * P,), F32, kind="ExternalInput")
    wv_d = nc.dram_tensor("wv", (C, 512), BF16, kind="ExternalInput")
    bv_d = nc.dram_tensor("bv", (1, 512), F32R, kind="ExternalInput")
    wproj_d = nc.dram_tensor("wproj", (8 * HS, C), BF16, kind="ExternalInput")
    y_d = nc.dram_tensor("y", (T, C), F32, kind="ExternalOutput")

    with tile.TileContext(nc) as tc, ExitStack() as big:
        const = big.enter_context(tc.tile_pool(name="const", bufs=1))
        persist = big.enter_context(tc.tile_pool(name="persist", bufs=1))

        # mask[k, q] = 1 where k <= q (valid causal entries of a diag block)
        mask = const.tile([P, P], BF16, tag="mask")
        make_upper_triangular(nc, mask[:], val=1.0, diag=True)
        ones_f = const.tile([P, P], F32, tag="ones_f")
        nc.vector.memset(ones_f[:], 1.0)
        ones_t = const.tile([1, P], F32R, tag="ones")
        nc.vector.tensor_copy(ones_t[:], ones_f[0:1, :])

        # qk_all: 8 blocks of [128, T] bf16; block 2p = q-pair of pair p
        # (rows 0:64 = head 2p channels, rows 64:128 = head 2p+1), block
        # 2p+1 = k-pair with the same row split.
        qk_all = persist.tile([P, 8 * T], BF16, tag="qk")

        # v_all: per (pair, kb): [vA(64) | onesA(1) | vB(64) | onesB(1)] = 130
        v_all = persist.tile([P, 4 * NKB * 130], BF16, tag="v")
        va4 = v_all[:].rearrange("p (a b c) -> p a b c", a=4, b=NKB, c=130)
        nc.vector.tensor_copy(va4[:, :, :, 64:65], ones_f[:, 0 : 4 * NKB])
        nc.vector.tensor_copy(va4[:, :, :, 129:130], ones_f[:, 0 : 4 * NKB])

        # aoT: pair-stacked [128 = ch(head 2p) | ch(head 2p+1), 4 * T] bf16
        aoT = persist.tile([P, 4 * T], BF16, tag="aoT")

        # sel2: rows {32p: cols 0:64 = 1}, {32p+1: cols 64:128 = 1}, else 0
        sel2 = const.tile([P, P], F32R, tag="sel2")
        nc.vector.memset(sel2[:].bitcast(F32), 0.0)
        for pr in range(4):
            nc.sync.dma_start(sel2[pr * 32 : pr * 32 + 1, 0:64].bitcast(F32), ones_f[0:1, 0:64])
            nc.sync.dma_start(
                sel2[pr * 32 + 1 : pr * 32 + 2, 64:P].bitcast(F32), ones_f[0:1, 0:64]
            )

        # wproj prefetch (DMA overlaps with phase 1 + attention)
        wpj = persist.tile([P, 4 * C], BF16, tag="wpj")
        nc.sync.dma_start(
            wpj[:].rearrange("r (pr j) -> r pr j", pr=4),
            wproj_d[:].rearrange("(pr r) j -> r pr j", r=P),
        )

        cp1 = big.enter_context(tc.tile_pool(name="cp1", bufs=1))
        bqk = cp1.tile([P, 8], F32, tag="bqk")
        nc.sync.dma_start(bqk[:], bqk_d[:].rearrange("(a p) -> p a", p=P))
        wv_sb = cp1.tile([P, 8 * 512], BF16, tag="wv")
        nc.sync.dma_start(
            wv_sb[:].rearrange("p (cb j) -> p cb j", cb=8),
            wv_d[:].rearrange("(cb p) j -> p cb j", p=P),
        )
        bvr = cp1.tile([1, 512], F32R, tag="bvr")
        nc.sync.dma_start(bvr[:], bv_d[:])
        bias_v = cp1.tile([P, 512], F32, tag="bias_v")

        # SBUF pools
        xtp = big.enter_context(tc.tile_pool(name="xtp", bufs=2))
        wp = big.enter_context(tc.tile_pool(name="wp", bufs=2))
        atp = big.enter_context(tc.tile_pool(name="atp", bufs=9))
        zrp = big.enter_context(tc.tile_pool(name="zrp", bufs=2))
        zsp_p = big.enter_context(tc.tile_pool(name="zsp_p", bufs=2))
        rzap = big.enter_context(tc.tile_pool(name="rzap", bufs=2))
        ysp = big.enter_context(tc.tile_pool(name="ysp", bufs=3))

        # PSUM pools: scores 2x[128,1024] = 4 banks, po0+po1 = 2 banks,
        # generic rotating [128,512] x2 = 2 banks.  Total 8.
        ps_s = big.enter_context(tc.tile_pool(name="ps_s", bufs=2, space="PSUM"))
        ps_o = big.enter_context(tc.tile_pool(name="ps_o", bufs=1, space="PSUM"))
        ps_g = big.enter_context(tc.tile_pool(name="ps_g", bufs=2, space="PSUM"))

        # bias_v[128, 512] = b_v broadcast along partitions (K=1 matmul)
        pbv = ps_g.tile([P, 512], F32, tag="g")
        nc.tensor.matmul(pbv[:], ones_t[:], bvr[:], start=True, stop=True)
        nc.vector.tensor_copy(bias_v[:], pbv[:])

        # ---------------- phase-1 emitters (qk^T + natural V) --------------
        def emit_p_qk(th, xT, chb):
            wb = wp.tile([P, 8 * P], BF16, tag="w")
            nc.sync.dma_start(
                wb[:].rearrange("p (cb j) -> p cb j", cb=8),
                wqk_d[:, chb * P : (chb + 1) * P].rearrange("(cb p) j -> p cb j", p=P),
            )
            for tck in range(2):
                pq = ps_g.tile([P, 512], F32, tag="g")
                for cb in range(8):
                    nc.tensor.matmul(
                        pq[:],
                        wb[:, cb * P : (cb + 1) * P],
                        xT[:, cb * TH + tck * 512 : cb * TH + (tck + 1) * 512],
                        start=(cb == 0),
                        stop=(cb == 7),
                    )
                t0 = th * TH + tck * 512
                nc.vector.tensor_scalar_add(
                    qk_all[:, chb * T + t0 : chb * T + t0 + 512],
                    pq[:],
                    bqk[:, chb : chb + 1],
                )

        def emit_p_v(th, xT, tb):
            kb = th * 8 + tb
            pv = ps_g.tile([P, 512], F32, tag="g")
            for cb in range(8):
                nc.tensor.matmul(
                    pv[:],
                    xT[:, cb * TH + tb * P : cb * TH + (tb + 1) * P],
                    wv_sb[:, cb * 512 : (cb + 1) * 512],
                    start=(cb == 0),
                    stop=(cb == 7),
                )
            dst = bass.AP(
                v_all[:].tensor,
                v_all[:].offset + kb * 130,
                [[v_all[:].ap[0][0], P], [NKB * 130, 4], [65, 2], [1, 64]],
            )
            src = bass.AP(
                pv[:].tensor,
                pv[:].offset,
                [[pv[:].ap[0][0], P], [128, 4], [64, 2], [1, 64]],
            )
            bsrc = bass.AP(
                bias_v[:].tensor,
                bias_v[:].offset,
                [[bias_v[:].ap[0][0], P], [128, 4], [64, 2], [1, 64]],
            )
            nc.vector.tensor_tensor(dst, src, bsrc, mybir.AluOpType.add)

        # ---------------- phase-2 emitters (attention + projection) --------
        def emit_score_batch(p_pair, qc, kbs):
            """Row-tiled score pairs + batched exp + causal mask for kbs."""
            qblk, kblk = 2 * p_pair, 2 * p_pair + 1
            q0 = qblk * T + qc * 512
            ats = {}
            for kb in kbs:
                qoff = max(0, kb * P - qc * 512)
                ps2 = ps_s.tile([P, 1024], F32, tag="ps")
                nc.tensor.matmul(
                    ps2[:, qoff:512],
                    qk_all[0:64, kblk * T + kb * P : kblk * T + (kb + 1) * P],
                    qk_all[0:64, q0 + qoff : q0 + 512],
                    start=True,
                    stop=True,
                )
                nc.tensor.matmul(
                    ps2[:, 512 + qoff : 1024],
                    qk_all[64:P, kblk * T + kb * P : kblk * T + (kb + 1) * P],
                    qk_all[64:P, q0 + qoff : q0 + 512],
                    start=True,
                    stop=True,
                )
                at2 = atp.tile([P, 1024], BF16, tag="at")
                if qoff == 0:
                    nc.scalar.activation(at2[:], ps2[:], Exp, scale=SCALE)
                else:
                    nc.scalar.activation(
                        at2[:, qoff:512], ps2[:, qoff:512], Exp, scale=SCALE
                    )
                    nc.scalar.activation(
                        at2[:, 512 + qoff : 1024], ps2[:, 512 + qoff : 1024],
                        Exp, scale=SCALE,
                    )
                if kb * P >= qc * 512:
                    # diagonal block: zero out k > q entries
                    nc.vector.tensor_mul(
                        at2[:, qoff : qoff + P], at2[:, qoff : qoff + P], mask[:]
                    )
                    nc.vector.tensor_mul(
                        at2[:, 512 + qoff : 512 + qoff + P],
                        at2[:, 512 + qoff : 512 + qoff + P],
                        mask[:],
                    )
                ats[kb] = (at2, qoff)
            return ats

        def emit_attnv_batch(p_pair, po0, po1, ats, nkb):
            for kb, (at2, qoff) in ats.items():
                base = p_pair * NKB * 130 + kb * 130
                nc.tensor.matmul(
                    po0[:, qoff:512],
                    v_all[:, base : base + 65],
                    at2[:, qoff:512],
                    start=(kb == 0),
                    stop=(kb == nkb - 1),
                    skip_group_check=True,
                )
                nc.tensor.matmul(
                    po1[:, qoff:512],
                    v_all[:, base + 65 : base + 130],
                    at2[:, 512 + qoff : 1024],
                    start=(kb == 0),
                    stop=(kb == nkb - 1),
                    skip_group_check=True,
                )

        def emit_round_tail(qc, zra, zrb, prs=(0, 1, 2, 3)):
            # spread Z rows across 128 partitions, reciprocal, unspread
            lo = min(prs) * 32
            hi = (max(prs) + 1) * 32
            zsp = zsp_p.tile([P, 32], F32, tag="zsp")
            for pr in prs:
                for hh in range(2):
                    r = pr * 2 + hh
                    srcz = (zra if hh == 0 else zrb)[pr * 32 : pr * 32 + 1, :]
                    nc.sync.dma_start(zsp[r * 16 : (r + 1) * 16, :], srcz)
            zspr = zsp_p.tile([P, 32], F32, tag="zspr")
            nc.vector.reciprocal(zspr[lo:hi, :], zsp[lo:hi, :])
            rz2 = rzap.tile([P, 512], F32R, tag="rz2")
            for pr in prs:
                for hh in range(2):
                    r = pr * 2 + hh
                    nc.sync.dma_start(
                        rz2[pr * 32 + hh : pr * 32 + hh + 1, :].bitcast(F32),
                        zspr[r * 16 : (r + 1) * 16, :],
                    )
            for pr in prs:
                col = pr * T + qc * 512
                pbt = ps_g.tile([P, 512], F32, tag="g")
                nc.tensor.matmul(
                    pbt[:],
                    sel2[pr * 32 : pr * 32 + 2, :],
                    rz2[pr * 32 : pr * 32 + 2, :],
                    start=True, stop=True,
                    tile_position=(pr * 32, 0),
                )
                nc.vector.tensor_mul(
                    aoT[0:64, col : col + 512],
                    aoT[0:64, col : col + 512],
                    pbt[0:64, :],
                )
                nc.vector.tensor_mul(
                    aoT[64:P, col : col + 512],
                    aoT[64:P, col : col + 512],
                    pbt[64:P, :],
                )

        def emit_proj_group(tb):
            for oc in range(2):
                py = ps_g.tile([P, 512], F32, tag="g")
                for pp in range(4):
                    nc.tensor.matmul(
                        py[:],
                        aoT[:, pp * T + tb * P : pp * T + (tb + 1) * P],
                        wpj[:, pp * C + oc * 512 : pp * C + (oc + 1) * 512],
                        start=(pp == 0),
                        stop=(pp == 3),
                    )
                ys = ysp.tile([P, 512], F32, tag="ys")
                nc.vector.tensor_copy(ys[:], py[:])
                nc.sync.dma_start(
                    y_d[tb * P : (tb + 1) * P, oc * 512 : (oc + 1) * 512],
                    ys[:],
                )

        # ---------------- emission schedule ----------------
        # Phase 1, first token half.
        xT0 = xtp.tile([P, 8 * TH], BF16, tag="xT")
        for cb in range(8):
            nc.sync.dma_start(
                xT0[:, cb * TH : (cb + 1) * TH], xt_d[cb * P : (cb + 1) * P, 0:TH]
            )
        for chb in range(8):
            emit_p_qk(0, xT0, chb)
        for tb in range(8):
            emit_p_v(0, xT0, tb)

        # Preload second-half xT; its projection work is emitted as fillers
        # between attention score batches of round qc=1.
        xT1 = xtp.tile([P, 8 * TH], BF16, tag="xT")
        for cb in range(8):
            nc.sync.dma_start(
                xT1[:, cb * TH : (cb + 1) * TH], xt_d[cb * P : (cb + 1) * P, TH:T]
            )
        fillers = []
        for chb in range(8):
            fillers.append(lambda chb=chb: emit_p_qk(1, xT1, chb))
        for tb in range(8):
            fillers.append(lambda tb=tb: emit_p_v(1, xT1, tb))

        pending = None
        for qc in range(NQC):
            zra = zrp.tile([P, 512], F32, tag="zra")
            zrb = zrp.tile([P, 512], F32, tag="zrb")
            for p_pair in range(4):
                po0 = ps_o.tile([65, 512], F32, tag="po0")
                po1 = ps_o.tile([65, 512], F32, tag="po1")
                nkb = 4 * qc + 4
                prev_ats = None
                for b0 in range(0, nkb, 4):
                    ats = emit_score_batch(p_pair, qc, range(b0, min(b0 + 4, nkb)))
                    if prev_ats is not None:
                        emit_attnv_batch(p_pair, po0, po1, prev_ats, nkb)
                    if qc == 1:
                        # drain 2 fillers per score batch (8 batches, 16 fillers)
                        for _ in range(2):
                            if fillers:
                                fillers.pop(0)()
                    prev_ats = ats
                emit_attnv_batch(p_pair, po0, po1, prev_ats, nkb)
                # evict raw ao + Z rows; normalization deferred one round
                col = p_pair * T + qc * 512
                nc.vector.tensor_copy(aoT[0:64, col : col + 512], po0[0:64, :])
                nc.vector.tensor_copy(aoT[64:P, col : col + 512], po1[0:64, :])
                nc.vector.tensor_copy(
                    zra[p_pair * 32 : p_pair * 32 + 1, :], po0[64:65, :]
                )
                nc.vector.tensor_copy(
                    zrb[p_pair * 32 : p_pair * 32 + 1, :], po1[64:65, :]
                )
                if qc == NQC - 1 and p_pair == 1:
                    emit_round_tail(qc, zra, zrb, prs=(0, 1))
            if pending is not None:
                emit_round_tail(*pending)
                for tb in range(pending[0] * 4, (pending[0] + 1) * 4):
                    emit_proj_group(tb)
            pending = (qc, zra, zrb)
        emit_round_tail(*pending, prs=(2, 3))
        for tb in range(12, 16):
            emit_proj_group(tb)

    nc.compile()
    return nc


def _shard_inputs(x, W_qkv, b_qkv, W_proj):
    """Build the 8 per-core input maps."""
    in_maps = []
    for c in range(8):
        b = c // 2
        hg = c % 2
        heads = [hg * 8 + j for j in range(8)]
        qk_cols = []
        for p in range(4):
            ha, hb = heads[2 * p], heads[2 * p + 1]
            for part in range(2):  # q, k
                qk_cols.extend(range(ha * 192 + part * 64, ha * 192 + part * 64 + 64))
                qk_cols.extend(range(hb * 192 + part * 64, hb * 192 + part * 64 + 64))
        qk_cols = np.array(qk_cols)
        v_cols = []
        for p in range(4):
            ha, hb = heads[2 * p], heads[2 * p + 1]
            v_cols.extend(range(ha * 192 + 128, ha * 192 + 192))
            v_cols.extend(range(hb * 192 + 128, hb * 192 + 192))
        v_cols = np.array(v_cols)
        in_maps.append(
            {
                "xt": np.ascontiguousarray(x[b].T.astype(ml_dtypes.bfloat16)),
                "wqk": np.ascontiguousarray(W_qkv[:, qk_cols].astype(ml_dtypes.bfloat16)),
                "bqk": np.ascontiguousarray(b_qkv[qk_cols], dtype=np.float32),
                "wv": np.ascontiguousarray(W_qkv[:, v_cols].astype(ml_dtypes.bfloat16)),
                "bv": np.ascontiguousarray(
                    b_qkv[v_cols].reshape(1, 512), dtype=np.float32
                ),
                "wproj": np.ascontiguousarray(
                    W_proj[hg * 512 : (hg + 1) * 512, :].astype(ml_dtypes.bfloat16)
                ),
            }
        )
    return in_maps


_NC = None


def kernel(x, W_qkv, b_qkv, W_proj, b_proj, _trace=False):
    global _NC
    x = np.asarray(x, dtype=np.float32)
    W_qkv = np.asarray(W_qkv, dtype=np.float32)
    b_qkv = np.asarray(b_qkv, dtype=np.float32)
    W_proj = np.asarray(W_proj, dtype=np.float32)
    b_proj = np.asarray(b_proj, dtype=np.float32)

    in_maps = _shard_inputs(x, W_qkv, b_qkv, W_proj)
    if _NC is None:
        _NC = build_kernel()
    res = run_bass_kernel_spmd(
        _NC, in_maps, core_ids=list(range(8)), trace=_trace,
        trace_cores=list(range(8)) if _trace else None,
    )
    out = np.empty((B, T, C), dtype=np.float32)
    for b in range(B):
        out[b] = res.results[2 * b]["y"] + res.results[2 * b + 1]["y"] + b_proj
    if _trace:
        return out, res
    return out


# revision 9
# speedup vs baseline: 1.0282x; 1.0282x over previous
"""Multi-head causal self-attention (B=4, T=2048, C=1024, H=16) on 8 TRN2 cores.

Sharding: core c handles batch b = c//2 and head-group hg = c%2 (8 heads):
data parallel over B, tensor parallel over H. Each core computes qk^T for its
heads, V in natural layout, causal attention for its 8 heads, and a partial
output projection (row-split W_proj) -> y_partial [T, C]. Host sums
y[b] = y_partial[2b] + y_partial[2b+1] + b_proj.

v2 over the original:
- q/k stored pair-packed [chA(64) | chB(64), T]; scores via two concurrent
  K=64 PE row-tiled matmuls (tile_position (0,0)/(64,0)) writing adjacent
  PSUM banks -> ~2x score throughput, no zero-padding memsets.
- exp batched over both heads' scores ([128,1024] PSUM read per ACTIVATE).
- score matmuls batched in groups of 4 kb-blocks to amortize PE tiling-mode
  switches vs the 128-row attnV matmuls.
- qkv projection for the second token half is emitted interleaved into
  attention round qc=1 (engines execute in-order; emission order = overlap).
- attention outputs kept in bf16 (aoT) and projection runs bf16 x bf16.
"""

from contextlib import ExitStack

import ml_dtypes
import numpy as np

import concourse.bass as bass
import concourse.bacc as bacc
import concourse.mybir as mybir
import concourse.tile as tile
from concourse.bass_utils import run_bass_kernel_spmd
from concourse.masks import make_upper_triangular

B, T, C, H, HS = 4, 2048, 1024, 16, 64
P = 128
NQC = T // 512          # q-chunks of 512
NKB = T // P            # key blocks of 128
TH = T // 2             # t-half
SCALE = HS ** -0.5

F32 = mybir.dt.float32
F32R = mybir.dt.float32r
BF16 = mybir.dt.bfloat16
Exp = mybir.ActivationFunctionType.Exp


def build_kernel():
    nc = bacc.Bacc("TRN2", target_bir_lowering=False)

    xt_d = nc.dram_tensor("xt", (C, T), BF16, kind="ExternalInput")
    wqk_d = nc.dram_tensor("wqk", (C, 8 * P), BF16, kind="ExternalInput")
    bqk_d = nc.dram_tensor("bqk", (8 * P,), F32, kind="ExternalInput")
    wv_d = nc.dram_tensor("wv", (C, 512), BF16, kind="ExternalInput")
    bv_d = nc.dram_tensor("bv", (1, 512), F32R, kind="ExternalInput")
    wproj_d = nc.dram_tensor("wproj", (8 * HS, C), BF16, kind="ExternalInput")
    y_d = nc.dram_tensor("y", (T, C), F32, kind="ExternalOutput")

    with tile.TileContext(nc) as tc, ExitStack() as big:
        const = big.enter_context(tc.tile_pool(name="const", bufs=1))
        persist = big.enter_context(tc.tile_pool(name="persist", bufs=1))

        # mask[k, q] = 1 where k <= q (valid causal entries of a diag block)
        mask = const.tile([P, P], BF16, tag="mask")
        make_upper_triangular(nc, mask[:], val=1.0, diag=True)
        ones_f = const.tile([P, P], F32, tag="ones_f")
        nc.vector.memset(ones_f[:], 1.0)
        ones_t = const.tile([1, P], F32R, tag="ones")
        nc.vector.tensor_copy(ones_t[:], ones_f[0:1, :])

        # qk_all: 8 blocks of [128, T] bf16; block 2p = q-pair of pair p
        # (rows 0:64 = head 2p channels, rows 64:128 = head 2p+1), block
        # 2p+1 = k-pair with the same row split.
        qk_all = persist.tile([P, 8 * T], BF16, tag="qk")

        # v_all: per (pair, kb): [vA(64) | onesA(1) | vB(64) | onesB(1)] = 130
        v_all = persist.tile([P, 4 * NKB * 130], BF16, tag="v")
        va4 = v_all[:].rearrange("p (a b c) -> p a b c", a=4, b=NKB, c=130)
        nc.vector.tensor_copy(va4[:, :, :, 64:65], ones_f[:, 0 : 4 * NKB])
        nc.vector.tensor_copy(va4[:, :, :, 129:130], ones_f[:, 0 : 4 * NKB])

        # aoT: pair-stacked [128 = ch(head 2p) | ch(head 2p+1), 4 * T] bf16
        aoT = persist.tile([P, 4 * T], BF16, tag="aoT")

        wpj = persist.tile([P, 4 * C], BF16, tag="wpj")

        cp1 = big.enter_context(tc.tile_pool(name="cp1", bufs=1))
        bqk = cp1.tile([P, 8], F32, tag="bqk")
        nc.sync.dma_start(bqk[:], bqk_d[:].rearrange("(a p) -> p a", p=P))
        wv_sb = cp1.tile([P, 8 * 512], BF16, tag="wv")
        nc.sync.dma_start(
            wv_sb[:].rearrange("p (cb j) -> p cb j", cb=8),
            wv_d[:].rearrange("(cb p) j -> p cb j", p=P),
        )
        bvr = cp1.tile([1, 512], F32R, tag="bvr")
        nc.sync.dma_start(bvr[:], bv_d[:])
        bias_v = cp1.tile([P, 512], F32, tag="bias_v")

        # SBUF pools
        xtp = big.enter_context(tc.tile_pool(name="xtp", bufs=2))
        wp = big.enter_context(tc.tile_pool(name="wp", bufs=2))
        wp1 = big.enter_context(tc.tile_pool(name="wp1", bufs=8))
        atp = big.enter_context(tc.tile_pool(name="atp", bufs=9))
        zrp = big.enter_context(tc.tile_pool(name="zrp", bufs=2))
        zsp_p = big.enter_context(tc.tile_pool(name="zsp_p", bufs=2))
        rzap = big.enter_context(tc.tile_pool(name="rzap", bufs=2))
        ysp = big.enter_context(tc.tile_pool(name="ysp", bufs=3))

        # PSUM pools: scores 2x[128,1024] = 4 banks, po0+po1 = 2 banks,
        # generic rotating [128,512] x2 = 2 banks.  Total 8.
        ps_s = big.enter_context(tc.tile_pool(name="ps_s", bufs=2, space="PSUM"))
        ps_o = big.enter_context(tc.tile_pool(name="ps_o", bufs=1, space="PSUM"))
        ps_g = big.enter_context(tc.tile_pool(name="ps_g", bufs=2, space="PSUM"))

        # PE warmup: keep TensorE busy during the initial DMA wait so the HAM
        # clock gate reaches 8/8 before real matmuls start.  Results unused.
        for _ in range(26):
            pw = ps_g.tile([P, 512], F32, tag="g")
            nc.tensor.matmul(pw[:, 0:P], mask[:], mask[:], start=True, stop=True)

        # bias_v[128, 512] = b_v broadcast along partitions (K=1 matmul)
        pbv = ps_g.tile([P, 512], F32, tag="g")
        nc.tensor.matmul(pbv[:], ones_t[:], bvr[:], start=True, stop=True)
        nc.vector.tensor_copy(bias_v[:], pbv[:])

        # ---------------- phase-1 emitters (qk^T + natural V) --------------
        def emit_p_qk(th, xT, chb, tcks=(0, 1), wpool=None, dma=True):
            if dma:
                wb = wp.tile([P, 8 * P], BF16, tag="w", name="wb") if wpool is None else wpool
                nc.sync.dma_start(
                    wb[:].rearrange("p (cb j) -> p cb j", cb=8),
                    wqk_d[:, chb * P : (chb + 1) * P].rearrange("(cb p) j -> p cb j", p=P),
                )
            else:
                wb = wpool
            for tck in tcks:
                pq = ps_g.tile([P, 512], F32, tag="g")
                for cb in range(8):
                    nc.tensor.matmul(
                        pq[:],
                        wb[:, cb * P : (cb + 1) * P],
                        xT[:, cb * TH + tck * 512 : cb * TH + (tck + 1) * 512],
                        start=(cb == 0),
                        stop=(cb == 7),
                    )
                t0 = th * TH + tck * 512
                nc.vector.tensor_scalar_add(
                    qk_all[:, chb * T + t0 : chb * T + t0 + 512],
                    pq[:],
                    bqk[:, chb : chb + 1],
                )

        def emit_p_v(th, xT, tb):
            kb = th * 8 + tb
            pv = ps_g.tile([P, 512], F32, tag="g")
            for cb in range(8):
                nc.tensor.matmul(
                    pv[:],
                    xT[:, cb * TH + tb * P : cb * TH + (tb + 1) * P],
                    wv_sb[:, cb * 512 : (cb + 1) * 512],
                    start=(cb == 0),
                    stop=(cb == 7),
                )
            dst = bass.AP(
                v_all[:].tensor,
                v_all[:].offset + kb * 130,
                [[v_all[:].ap[0][0], P], [NKB * 130, 4], [65, 2], [1, 64]],
            )
            src = bass.AP(
                pv[:].tensor,
                pv[:].offset,
                [[pv[:].ap[0][0], P], [128, 4], [64, 2], [1, 64]],
            )
            bsrc = bass.AP(
                bias_v[:].tensor,
                bias_v[:].offset,
                [[bias_v[:].ap[0][0], P], [128, 4], [64, 2], [1, 64]],
            )
            nc.vector.tensor_tensor(dst, src, bsrc, mybir.AluOpType.add)

        # ---------------- phase-2 emitters (attention + projection) --------
        def emit_score_batch(p_pair, qc, kbs):
            """Row-tiled score pairs + batched exp + causal mask for kbs.

            Head A scores land at ps2[qoff:512], head B at
            ps2[512 : 1024-qoff] (left-aligned to the bank boundary) so one
            contiguous ACTIVATE covers both heads even for diagonal blocks.
            """
            qblk, kblk = 2 * p_pair, 2 * p_pair + 1
            q0 = qblk * T + qc * 512
            ats = {}
            for kb in kbs:
                qoff = max(0, kb * P - qc * 512)
                ps2 = ps_s.tile([P, 1024], F32, tag="ps")
                nc.tensor.matmul(
                    ps2[:, qoff:512],
                    qk_all[0:64, kblk * T + kb * P : kblk * T + (kb + 1) * P],
                    qk_all[0:64, q0 + qoff : q0 + 512],
                    start=True,
                    stop=True,
                )
                nc.tensor.matmul(
                    ps2[:, 512 : 1024 - qoff],
                    qk_all[64:P, kblk * T + kb * P : kblk * T + (kb + 1) * P],
                    qk_all[64:P, q0 + qoff : q0 + 512],
                    start=True,
                    stop=True,
                )
                at2 = atp.tile([P, 1024], BF16, tag="at")
                nc.scalar.activation(
                    at2[:, qoff : 1024 - qoff], ps2[:, qoff : 1024 - qoff],
                    Exp, scale=SCALE,
                )
                if kb * P >= qc * 512:
                    # diagonal block: zero out k > q entries
                    nc.vector.tensor_mul(
                        at2[:, qoff : qoff + P], at2[:, qoff : qoff + P], mask[:]
                    )
                    nc.vector.tensor_mul(
                        at2[:, 512 : 512 + P], at2[:, 512 : 512 + P], mask[:]
                    )
                ats[kb] = (at2, qoff)
            return ats

        def emit_attnv_batch(p_pair, po0, po1, ats, nkb):
            for kb, (at2, qoff) in ats.items():
                base = p_pair * NKB * 130 + kb * 130
                nc.tensor.matmul(
                    po0[:, qoff:512],
                    v_all[:, base : base + 65],
                    at2[:, qoff:512],
                    start=(kb == 0),
                    stop=(kb == nkb - 1),
                    skip_group_check=True,
                )
                nc.tensor.matmul(
                    po1[:, qoff:512],
                    v_all[:, base + 65 : base + 130],
                    at2[:, 512 : 1024 - qoff],
                    start=(kb == 0),
                    stop=(kb == nkb - 1),
                    skip_group_check=True,
                )

        def emit_round_tail(qc, zra, zrb, prs=(0, 1, 2, 3)):
            # spread Z rows across 128 partitions, reciprocal, unspread
            lo = min(prs) * 32
            hi = (max(prs) + 1) * 32
            zsp = zsp_p.tile([P, 32], F32, tag="zsp")
            for pr in prs:
                for hh in range(2):
                    r = pr * 2 + hh
                    srcz = (zra if hh == 0 else zrb)[pr * 32 : pr * 32 + 1, :]
                    nc.sync.dma_start(zsp[r * 16 : (r + 1) * 16, :], srcz)
            zspr = zsp_p.tile([P, 32], F32, tag="zspr")
            nc.vector.reciprocal(zspr[lo:hi, :], zsp[lo:hi, :])
            rz2 = rzap.tile([P, 512], F32R, tag="rz2")
            for pr in prs:
                for hh in range(2):
                    r = pr * 2 + hh
                    nc.sync.dma_start(
                        rz2[pr * 32 + hh : pr * 32 + hh + 1, :].bitcast(F32),
                        zspr[r * 16 : (r + 1) * 16, :],
                    )
            for pr in prs:
                col = pr * T + qc * 512
                pbt = ps_g.tile([P, 512], F32, tag="g")
                nc.tensor.matmul(
                    pbt[:],
                    sel2[pr * 32 : pr * 32 + 2, :],
                    rz2[pr * 32 : pr * 32 + 2, :],
                    start=True, stop=True,
                    tile_position=(pr * 32, 0),
                )
                nc.vector.tensor_mul(
                    aoT[0:64, col : col + 512],
                    aoT[0:64, col : col + 512],
                    pbt[0:64, :],
                )
                nc.vector.tensor_mul(
                    aoT[64:P, col : col + 512],
                    aoT[64:P, col : col + 512],
                    pbt[64:P, :],
                )

        def emit_proj_group(tb):
            for oc in range(2):
                py = ps_g.tile([P, 512], F32, tag="g")
                for pp in range(4):
                    nc.tensor.matmul(
                        py[:],
                        aoT[:, pp * T + tb * P : pp * T + (tb + 1) * P],
                        wpj[:, pp * C + oc * 512 : pp * C + (oc + 1) * 512],
                        start=(pp == 0),
                        stop=(pp == 3),
                    )
                ys = ysp.tile([P, 512], F32, tag="ys")
                nc.vector.tensor_copy(ys[:], py[:])
                nc.sync.dma_start(
                    y_d[tb * P : (tb + 1) * P, oc * 512 : (oc + 1) * 512],
                    ys[:],
                )

        # ---------------- emission schedule ----------------
        # Phase 1, first token half.  xT split into 16 DMAs to use all queues.
        xT0 = xtp.tile([P, 8 * TH], BF16, tag="xT")
        for cb in range(8):
            for hf in range(2):
                nc.sync.dma_start(
                    xT0[:, cb * TH + hf * 512 : cb * TH + (hf + 1) * 512],
                    xt_d[cb * P : (cb + 1) * P, hf * 512 : (hf + 1) * 512],
                )
        for chb in range(8):
            emit_p_qk(0, xT0, chb)
        for tb in range(8):
            emit_p_v(0, xT0, tb)

        # Preload second-half xT + P1 weights + wproj; the second-half
        # projection work is emitted as fillers inside rounds qc=1 / qc=2.
        xT1 = xtp.tile([P, 8 * TH], BF16, tag="xT")
        for cb in range(8):
            for hf in range(2):
                nc.sync.dma_start(
                    xT1[:, cb * TH + hf * 512 : cb * TH + (hf + 1) * 512],
                    xt_d[cb * P : (cb + 1) * P, TH + hf * 512 : TH + (hf + 1) * 512],
                )
        wb1 = []
        for chb in range(8):
            wb = wp1.tile([P, 8 * P], BF16, tag="w1")
            nc.sync.dma_start(
                wb[:].rearrange("p (cb j) -> p cb j", cb=8),
                wqk_d[:, chb * P : (chb + 1) * P].rearrange("(cb p) j -> p cb j", p=P),
            )
            wb1.append(wb)
        for pr in range(4):
            nc.sync.dma_start(
                wpj[:, pr * C : (pr + 1) * C],
                wproj_d[pr * P : (pr + 1) * P, :],
            )

        # sel2: rows {32p: cols 0:64 = 1}, {32p+1: cols 64:128 = 1}, else 0
        sel2 = const.tile([P, P], F32R, tag="sel2")
        nc.vector.memset(sel2[:].bitcast(F32), 0.0)
        for pr in range(4):
            nc.sync.dma_start(sel2[pr * 32 : pr * 32 + 1, 0:64].bitcast(F32), ones_f[0:1, 0:64])
            nc.sync.dma_start(
                sel2[pr * 32 + 1 : pr * 32 + 2, 64:P].bitcast(F32), ones_f[0:1, 0:64]
            )

        # Filler queues of dense 128-row-mode tensor work, drained between
        # attention batches of the given round (keeps TensorE fed while
        # ScalarE chews exps).  P1a (tokens 1024:1536) must finish before
        # qc=2; P1b (1536:2048) before qc=3.
        fillers = {1: [], 2: [], 3: []}
        for chb in range(8):
            fillers[1].append(
                lambda chb=chb: emit_p_qk(1, xT1, chb, tcks=(0,), wpool=wb1[chb], dma=False)
            )
        for tb in range(4):
            fillers[1].append(lambda tb=tb: emit_p_v(1, xT1, tb))
        for chb in range(8):
            fillers[2].append(
                lambda chb=chb: emit_p_qk(1, xT1, chb, tcks=(1,), wpool=wb1[chb], dma=False)
            )
        for tb in range(4, 8):
            fillers[2].append(lambda tb=tb: emit_p_v(1, xT1, tb))

        def drain(qc, n):
            q = fillers.get(qc)
            for _ in range(n):
                if q:
                    q.pop(0)()

        # batches per round: qc=1: 8, qc=2: 12, qc=3: 16
        per_batch = {1: 2, 2: 2, 3: 1}

        pending = None
        for qc in range(NQC):
            zra = zrp.tile([P, 512], F32, tag="zra")
            zrb = zrp.tile([P, 512], F32, tag="zrb")
            for p_pair in range(4):
                po0 = ps_o.tile([65, 512], F32, tag="po0")
                po1 = ps_o.tile([65, 512], F32, tag="po1")
                nkb = 4 * qc + 4
                prev_ats = None
                for b0 in range(0, nkb, 4):
                    ats = emit_score_batch(p_pair, qc, range(b0, min(b0 + 4, nkb)))
                    if prev_ats is not None:
                        emit_attnv_batch(p_pair, po0, po1, prev_ats, nkb)
                    drain(qc, per_batch.get(qc, 0))
                    prev_ats = ats
                emit_attnv_batch(p_pair, po0, po1, prev_ats, nkb)
                # evict raw ao + Z rows; normalization deferred ~one round
                col = p_pair * T + qc * 512
                nc.vector.tensor_copy(aoT[0:64, col : col + 512], po0[0:64, :])
                nc.vector.tensor_copy(aoT[64:P, col : col + 512], po1[0:64, :])
                nc.vector.tensor_copy(
                    zra[p_pair * 32 : p_pair * 32 + 1, :], po0[64:65, :]
                )
                nc.vector.tensor_copy(
                    zrb[p_pair * 32 : p_pair * 32 + 1, :], po1[64:65, :]
                )
                if p_pair == 0 and pending is not None:
                    # normalize previous round (Z DMA latency hidden behind
                    # pair 0's compute) and queue its projection as fillers
                    emit_round_tail(*pending)
                    for tb in range(pending[0] * 4, (pending[0] + 1) * 4):
                        fillers[qc].append(lambda tb=tb: emit_proj_group(tb))
                    pending = None
                if qc == NQC - 1 and p_pair == 1:
                    emit_round_tail(qc, zra, zrb, prs=(0, 1))
            drain(qc, len(fillers.get(qc, ())))
            pending = (qc, zra, zrb)
        emit_round_tail(*pending, prs=(2, 3))
        for tb in range(12, 16):
            emit_proj_group(tb)

    nc.compile()
    return nc


def _shard_inputs(x, W_qkv, b_qkv, W_proj):
    """Build the 8 per-core input maps."""
    in_maps = []
    for c in range(8):
        b = c // 2
        hg = c % 2
        heads = [hg * 8 + j for j in range(8)]
        qk_cols = []
        for p in range(4):
            ha, hb = heads[2 * p], heads[2 * p + 1]
            for part in range(2):  # q, k
                qk_cols.extend(range(ha * 192 + part * 64, ha * 192 + part * 64 + 64))
                qk_cols.extend(range(hb * 192 + part * 64, hb * 192 + part * 64 + 64))
        qk_cols = np.array(qk_cols)
        v_cols = []
        for p in range(4):
            ha, hb = heads[2 * p], heads[2 * p + 1]
            v_cols.extend(range(ha * 192 + 128, ha * 192 + 192))
            v_cols.extend(range(hb * 192 + 128, hb * 192 + 192))
        v_cols = np.array(v_cols)
        in_maps.append(
            {
                "xt": np.ascontiguousarray(x[b].T.astype(ml_dtypes.bfloat16)),
                "wqk": np.ascontiguousarray(W_qkv[:, qk_cols].astype(ml_dtypes.bfloat16)),
                "bqk": np.ascontiguousarray(b_qkv[qk_cols], dtype=np.float32),
                "wv": np.ascontiguousarray(W_qkv[:, v_cols].astype(ml_dtypes.bfloat16)),
                "bv": np.ascontiguousarray(
                    b_qkv[v_cols].reshape(1, 512), dtype=np.float32
                ),
                "wproj": np.ascontiguousarray(
                    W_proj[hg * 512 : (hg + 1) * 512, :].astype(ml_dtypes.bfloat16)
                ),
            }
        )
    return in_maps


_NC = None


def kernel(x, W_qkv, b_qkv, W_proj, b_proj, _trace=False):
    global _NC
    x = np.asarray(x, dtype=np.float32)
    W_qkv = np.asarray(W_qkv, dtype=np.float32)
    b_qkv = np.asarray(b_qkv, dtype=np.float32)
    W_proj = np.asarray(W_proj, dtype=np.float32)
    b_proj = np.asarray(b_proj, dtype=np.float32)

    in_maps = _shard_inputs(x, W_qkv, b_qkv, W_proj)
    if _NC is None:
        _NC = build_kernel()
    res = run_bass_kernel_spmd(
        _NC, in_maps, core_ids=list(range(8)), trace=_trace,
        trace_cores=list(range(8)) if _trace else None,
    )
    out = np.empty((B, T, C), dtype=np.float32)
    for b in range(B):
        out[b] = res.results[2 * b]["y"] + res.results[2 * b + 1]["y"] + b_proj
    if _trace:
        return out, res
    return out
